# revision 3
# baseline (speedup 1.0000x reference)
"""Trainium2 Bass kernel for nn_AttentionNet (topk_masking), 8 NeuronCores.

Pipeline (per the row-sharded strategy):
  L1 (device): per-core scores A_i = w2 . tanh(W1 x_i + b1) for its row shard.
      x is fed pre-transposed (xT shard, [D, rows]) so the D-contraction sits on
      the partition axis; matmuls run in float32r (full-rate fp32 path).
  Host: global top-k selection. float32r carries ~1.5e-4 relative error, so rows
      within a margin of the k-th score are re-scored exactly (fp64) to pin the
      exact membership; exp/normalize gives the Ak output and the masked weight
      vector for the weighted sum.
  L2 (device): per-core masked weighted sum M_partial = sum_i w_i x_i over its
      row shard (natural layout, rows on the contraction axis), AllReduce across
      the 8 cores -> M.

Outputs match the reference tuple: (M [2048] f32, Ak [5000,1] f32).
"""
import numpy as np

from concourse import bacc, bass, mybir, tile
from concourse.bass_utils import run_bass_kernel_spmd

# problem constants (hardcoded per harness contract)
N, D, H = 50000, 2048, 512
K_TOP = 5000
NC = 8
SHARD = N // NC            # 6250 rows per core
CHUNK = 448                # moving-dim chunk (>=256 keeps float32r at full rate)
NCHUNK = 14
PAD = CHUNK * NCHUNK       # 6272 = 49*128
KT = D // 128              # 16 contraction tiles
HT = H // 128              # 4 hidden tiles
RCHUNKS = (SHARD + 127) // 128   # 49 row chunks in natural layout (last ragged)

f32 = mybir.dt.float32
f32r = mybir.dt.float32r
AF = mybir.ActivationFunctionType

_programs = {}
LAST = {}  # test harness introspection: {"l1": BassKernelResults, "l2": ...}


def _build_l1():
    """Scores kernel: a_out[0, i] = sum_h w2[h] * tanh(sum_d W1[h,d] x[i,d] + b1[h])."""
    nc = bacc.Bacc(None, target_bir_lowering=False, debug=False)
    xt_d = nc.dram_tensor("xt", [D, PAD], f32r, kind="ExternalInput")
    w1t_d = nc.dram_tensor("w1t", [D, H], f32r, kind="ExternalInput")
    b1_d = nc.dram_tensor("b1c", [H, 1], f32, kind="ExternalInput")
    w2_d = nc.dram_tensor("w2c", [H, 1], f32r, kind="ExternalInput")
    a_d = nc.dram_tensor("a_out", [1, PAD], f32, kind="ExternalOutput")

    with tile.TileContext(nc) as tc:
        with (
            tc.tile_pool(name="wpool", bufs=1) as wpool,
            tc.tile_pool(name="xpool", bufs=2) as xpool,
            tc.tile_pool(name="tpool", bufs=6) as tpool,
            tc.tile_pool(name="apool", bufs=1) as apool,
            tc.tile_pool(name="zpsum", bufs=2, space="PSUM") as zpsum,
            tc.tile_pool(name="apsum", bufs=2, space="PSUM") as apsum,
        ):
            w1_sb = wpool.tile([128, KT, H], f32r)
            nc.sync.dma_start(
                w1_sb[:], w1t_d.ap().rearrange("(k p) h -> p k h", p=128)
            )
            b1_sb = wpool.tile([128, HT], f32)
            nc.sync.dma_start(
                b1_sb[:], b1_d.ap().rearrange("(h p) o -> p (h o)", p=128)
            )
            w2_sb = wpool.tile([128, HT], f32r)
            nc.sync.dma_start(
                w2_sb[:], w2_d.ap().rearrange("(h p) o -> p (h o)", p=128)
            )
            a_sb = apool.tile([1, PAD], f32)

            xt_view = xt_d.ap().rearrange("(k p) (c n) -> p k c n", p=128, n=CHUNK)
            for c in range(NCHUNK):
                xt_sb = xpool.tile([128, KT, CHUNK], f32r, tag="xt")
                nc.sync.dma_start(xt_sb[:], xt_view[:, :, c, :])
                t_tiles = []
                for h in range(HT):
                    z_ps = zpsum.tile([128, CHUNK], f32, tag="z")
                    for k in range(KT):
                        nc.tensor.matmul(
                            z_ps[:],
                            w1_sb[:, k, h * 128:(h + 1) * 128],
                            xt_sb[:, k, :],
                            start=(k == 0),
                            stop=(k == KT - 1),
                        )
                    t_sb = tpool.tile([128, CHUNK], f32r, tag="t")
                    nc.scalar.activation(
                        t_sb[:], z_ps[:], AF.Tanh, bias=b1_sb[:, h:h + 1]
                    )
                    t_tiles.append(t_sb)
                a_ps = apsum.tile([1, CHUNK], f32, tag="a")
                for h in range(HT):
                    nc.tensor.matmul(
                        a_ps[:],
                        w2_sb[:, h:h + 1],
                        t_tiles[h][:],
                        start=(h == 0),
                        stop=(h == HT - 1),
                    )
                nc.scalar.copy(a_sb[:, c * CHUNK:(c + 1) * CHUNK], a_ps[:])
            nc.sync.dma_start(a_d[:], a_sb[:])
    nc.compile()
    return nc


def _build_l2():
    """Masked weighted sum: m = AllReduce_j( sum_i wv[i] * x_shard_j[i, :] )."""
    nc = bacc.Bacc(None, target_bir_lowering=False, debug=False)
    x_d = nc.dram_tensor("xs", [SHARD, D], f32r, kind="ExternalInput")
    w_d = nc.dram_tensor("wv", [PAD, 1], f32r, kind="ExternalInput")
    m_d = nc.dram_tensor("m_out", [1, D], f32, kind="ExternalOutput")

    with tile.TileContext(nc) as tc:
        with (
            tc.tile_pool(name="wpool", bufs=1) as wpool,
            tc.tile_pool(name="xpool", bufs=4) as xpool,
            tc.tile_pool(name="spool", bufs=1) as spool,
            tc.tile_pool(name="mpsum", bufs=1, space="PSUM") as mpsum,
            tc.tile_pool(name="dram", bufs=1, space="DRAM") as dram,
        ):
            w_sb = wpool.tile([128, RCHUNKS], f32r)
            nc.scalar.dma_start(
                w_sb[:], w_d.ap().rearrange("(c p) o -> p (c o)", p=128)
            )
            m_ps = [
                mpsum.tile([1, 512], f32, tag=f"m{d4}", name=f"m_ps{d4}")
                for d4 in range(4)
            ]
            for c in range(RCHUNKS):
                rows = min(128, SHARD - c * 128)
                xg = xpool.tile([128, D], f32r, tag="xg")
                nc.scalar.dma_start(xg[:rows, :], x_d[c * 128:c * 128 + rows, :])
                for d4 in range(4):
                    nc.tensor.matmul(
                        m_ps[d4][:],
                        w_sb[:rows, c:c + 1],
                        xg[:rows, d4 * 512:(d4 + 1) * 512],
                        start=(c == 0),
                        stop=(c == RCHUNKS - 1),
                    )
            m_sb = spool.tile([1, D], f32)
            for d4 in range(4):
                nc.scalar.copy(m_sb[:, d4 * 512:(d4 + 1) * 512], m_ps[d4][:])
            m_bin = dram.tile([1, D], f32)
            m_bout = dram.tile([1, D], f32)
            nc.gpsimd.dma_start(m_bin[:], m_sb[:])
            nc.gpsimd.collective_compute(
                "AllReduce",
                mybir.AluOpType.add,
                replica_groups=[list(range(NC))],
                ins=[m_bin.opt()],
                outs=[m_bout.opt()],
            )
            nc.gpsimd.dma_start(m_d[:], m_bout[:])
    nc.compile()
    return nc


def _program(name, builder):
    if name not in _programs:
        _programs[name] = builder()
    return _programs[name]


def kernel(x, W1, b1, W2, b2):
    x = np.ascontiguousarray(np.asarray(x), dtype=np.float32)
    W1 = np.ascontiguousarray(np.asarray(W1), dtype=np.float32)
    b1 = np.asarray(b1, dtype=np.float32).reshape(H)
    W2 = np.asarray(W2, dtype=np.float32).reshape(1, H)
    b2v = float(np.asarray(b2, dtype=np.float32).reshape(1)[0])

    # ---- L1: scores on device ----
    l1 = _program("l1", _build_l1)
    w1t = np.ascontiguousarray(W1.T)              # [D, H]
    b1c = np.ascontiguousarray(b1.reshape(H, 1))
    w2c = np.ascontiguousarray(W2.T)              # [H, 1]
    xT = x.T                                      # view [D, N]
    in1 = []
    for j in range(NC):
        sh = np.zeros((D, PAD), np.float32)
        sh[:, :SHARD] = xT[:, j * SHARD:(j + 1) * SHARD]
        in1.append({"xt": sh, "w1t": w1t, "b1c": b1c, "w2c": w2c})
    res1 = run_bass_kernel_spmd(l1, in1, list(range(NC)))
    LAST["l1"] = res1
    A = np.concatenate(
        [res1.results[j]["a_out"][0, :SHARD] for j in range(NC)]
    ).astype(np.float64)
    A += b2v

    # ---- host: exact top-k membership (fp64 re-score near the cut), Ak ----
    order = np.argsort(-A, kind="stable")
    tau = A[order[K_TOP - 1]]
    MARGIN = 6e-3   # ~40 sigma of the float32r score error
    cand = np.where(np.abs(A - tau) <= MARGIN)[0]
    if cand.size:
        xa = x[cand].astype(np.float64)
        A[cand] = (
            np.tanh(xa @ W1.T.astype(np.float64) + b1.astype(np.float64))
            @ W2[0].astype(np.float64) + b2v
        )
        order = np.argsort(-A, kind="stable")
    sel = order[:K_TOP]
    e = np.exp(A[sel] - A.max())
    w_norm = e / e.sum()
    Ak = w_norm.astype(np.float32).reshape(K_TOP, 1)

    # ---- L2: masked weighted sum + AllReduce on device ----
    wfull = np.zeros(N, np.float64)
    wfull[sel] = w_norm
    l2 = _program("l2", _build_l2)
    in2 = []
    for j in range(NC):
        wv = np.zeros((PAD, 1), np.float32)
        wv[:SHARD, 0] = wfull[j * SHARD:(j + 1) * SHARD]
        in2.append({"xs": x[j * SHARD:(j + 1) * SHARD], "wv": wv})
    res2 = run_bass_kernel_spmd(l2, in2, list(range(NC)))
    LAST["l2"] = res2
    M = np.ascontiguousarray(res2.results[0]["m_out"][0], dtype=np.float32)
    return (M, Ak)


# revision 5
# speedup vs baseline: 1.5249x; 1.5249x over previous
"""Trainium2 Bass kernel for nn_AttentionNet (topk_masking), 8 NeuronCores.

Pipeline (row-sharded across 8 cores):
  L1 (device): per-core scores A_i = w2 . tanh(W1 x_i + b1) for its row shard.
      x is fed pre-transposed (xT shard, [D, rows]) so the D-contraction sits on
      the partition axis; matmuls run in float32r (full-rate fp32 path).
  Host: global top-k selection. float32r carries ~1.5e-4 relative error, so rows
      within a margin of the k-th score are re-scored exactly (fp64) to pin the
      exact membership; exp/normalize gives the Ak output and per-core gather
      lists for the weighted sum.
  L2 (device): per-core gather of its selected rows (indirect DMA) + weighted
      sum matmul, AllReduce across the 8 cores -> M.

Outputs match the reference tuple: (M [2048] f32, Ak [5000,1] f32).
"""
import numpy as np

from concourse import bacc, bass, mybir, tile
from concourse.bass_utils import run_bass_kernel_spmd

# problem constants (hardcoded per harness contract)
N, D, H = 50000, 2048, 512
K_TOP = 5000
NC = 8
SHARD = N // NC            # 6250 rows per core
CHUNK = 448                # moving-dim chunk (>=256 keeps float32r at full rate)
NCHUNK = 14
PAD = CHUNK * NCHUNK       # 6272 = 49*128
KT = D // 128              # 16 contraction tiles
HT = H // 128              # 4 hidden tiles
KG = 1024                  # gathered rows per core in L2 (mean 625, ~17 sigma pad)

f32 = mybir.dt.float32
f32r = mybir.dt.float32r
i32 = mybir.dt.int32
AF = mybir.ActivationFunctionType

_programs = {}
LAST = {}  # test harness introspection: {"l1": BassKernelResults, "l2": ...}


def _build_l1():
    """Scores kernel: a_out[0, i] = sum_h w2[h] * tanh(sum_d W1[h,d] x[i,d] + b1[h])."""
    nc = bacc.Bacc(None, target_bir_lowering=False, debug=False)
    xt_d = nc.dram_tensor("xt", [D, PAD], f32r, kind="ExternalInput")
    w1t_d = nc.dram_tensor("w1t", [D, H], f32r, kind="ExternalInput")
    b1_d = nc.dram_tensor("b1c", [H, 1], f32, kind="ExternalInput")
    w2_d = nc.dram_tensor("w2c", [H, 1], f32r, kind="ExternalInput")
    a_d = nc.dram_tensor("a_out", [1, PAD], f32, kind="ExternalOutput")

    with tile.TileContext(nc) as tc:
        with (
            tc.tile_pool(name="wpool", bufs=1) as wpool,
            tc.tile_pool(name="xpool", bufs=2) as xpool,
            tc.tile_pool(name="tpool", bufs=10) as tpool,
            tc.tile_pool(name="apool", bufs=1) as apool,
            tc.tile_pool(name="zpsum", bufs=2, space="PSUM") as zpsum,
            tc.tile_pool(name="apsum", bufs=2, space="PSUM") as apsum,
            tc.tile_pool(name="wupsum", bufs=1, space="PSUM") as wupsum,
        ):
            # PE warmup: trip the HAM clock gate to 2.4 GHz while the first
            # DMAs land. Scratch inputs are zeroed; results never read.
            wu_sb = wpool.tile([128, 256], mybir.dt.bfloat16)
            nc.vector.memset(wu_sb[:], 0.0)
            wu_ps = wupsum.tile([128, 256], f32)
            for _ in range(16):
                nc.tensor.matmul(wu_ps[:], wu_sb[:, :128], wu_sb[:], start=True, stop=True)

            # weights: one tile per contraction slice so the first matmul only
            # waits on its own 256 KB slice
            w1_k = []
            w1t_view = w1t_d.ap().rearrange("(k p) h -> p k h", p=128)
            for k in range(KT):
                wk = wpool.tile([128, H], f32r, name=f"w1_{k}", tag=f"w1_{k}")
                nc.sync.dma_start(wk[:], w1t_view[:, k, :])
                w1_k.append(wk)
            b1_sb = wpool.tile([128, HT], f32)
            nc.sync.dma_start(b1_sb[:], b1_d.ap().rearrange("(h p) o -> p (h o)", p=128))
            w2_sb = wpool.tile([128, HT], f32r)
            nc.sync.dma_start(w2_sb[:], w2_d.ap().rearrange("(h p) o -> p (h o)", p=128))
            a_sb = apool.tile([1, PAD], f32)

            xt_view = xt_d.ap().rearrange("(k p) (c n) -> p k c n", p=128, n=CHUNK)
            pending = None  # (t_tiles, chunk_idx) whose W2-dot is deferred
            for c in range(NCHUNK):
                xt_k = []
                for k in range(KT):
                    xk = xpool.tile([128, CHUNK], f32r, name=f"xt_{k}", tag=f"xt_{k}")
                    nc.sync.dma_start(xk[:], xt_view[:, k, c, :])
                    xt_k.append(xk)
                t_tiles = []
                for h in range(HT):
                    z_ps = zpsum.tile([128, CHUNK], f32, tag="z")
                    for k in range(KT):
                        nc.tensor.matmul(
                            z_ps[:],
                            w1_k[k][:, h * 128:(h + 1) * 128],
                            xt_k[k][:],
                            start=(k == 0),
                            stop=(k == KT - 1),
                        )
                    t_sb = tpool.tile([128, CHUNK], f32r, tag="t")
                    nc.scalar.activation(t_sb[:], z_ps[:], AF.Tanh, bias=b1_sb[:, h:h + 1])
                    t_tiles.append(t_sb)
                # W2-dot for the PREVIOUS chunk: by now its tanh tiles are long
                # done, so the PE never stalls on ScalarE
                if pending is not None:
                    _emit_w2_dot(nc, apsum, w2_sb, a_sb, *pending)
                pending = (t_tiles, c)
            _emit_w2_dot(nc, apsum, w2_sb, a_sb, *pending)
            nc.sync.dma_start(a_d[:], a_sb[:])
    nc.compile()
    return nc


def _emit_w2_dot(nc, apsum, w2_sb, a_sb, t_tiles, c):
    a_ps = apsum.tile([1, CHUNK], f32, tag="a", name=f"a_ps{c}")
    for h in range(HT):
        nc.tensor.matmul(
            a_ps[:], w2_sb[:, h:h + 1], t_tiles[h][:],
            start=(h == 0), stop=(h == HT - 1),
        )
    nc.scalar.copy(a_sb[:, c * CHUNK:(c + 1) * CHUNK], a_ps[:])


def _build_l2():
    """Weighted sum of gathered rows: m = AllReduce_j( sum_i gw[i] * xs[gidx[i], :] )."""
    nc = bacc.Bacc(None, target_bir_lowering=False, debug=False)
    x_d = nc.dram_tensor("xs", [SHARD, D], f32r, kind="ExternalInput")
    i_d = nc.dram_tensor("gidx", [KG, 1], i32, kind="ExternalInput")
    w_d = nc.dram_tensor("gw", [KG, 1], f32r, kind="ExternalInput")
    m_d = nc.dram_tensor("m_out", [1, D], f32, kind="ExternalOutput")
    CG = KG // 128  # gather chunks

    with tile.TileContext(nc) as tc:
        with (
            tc.tile_pool(name="wpool", bufs=1) as wpool,
            tc.tile_pool(name="xpool", bufs=4) as xpool,
            tc.tile_pool(name="spool", bufs=1) as spool,
            tc.tile_pool(name="mpsum", bufs=1, space="PSUM") as mpsum,
            tc.tile_pool(name="dram", bufs=1, space="DRAM") as dram,
        ):
            idx_sb = wpool.tile([128, CG], i32)
            nc.scalar.dma_start(idx_sb[:], i_d.ap().rearrange("(c p) o -> p (c o)", p=128))
            w_sb = wpool.tile([128, CG], f32r)
            nc.scalar.dma_start(w_sb[:], w_d.ap().rearrange("(c p) o -> p (c o)", p=128))
            m_ps = [
                mpsum.tile([1, 512], f32, tag=f"m{d4}", name=f"m_ps{d4}")
                for d4 in range(4)
            ]
            for c in range(CG):
                xg = xpool.tile([128, D], f32r, tag="xg")
                nc.gpsimd.indirect_dma_start(
                    out=xg[:],
                    out_offset=None,
                    in_=x_d[:],
                    in_offset=bass.IndirectOffsetOnAxis(ap=idx_sb[:, c:c + 1], axis=0),
                )
                for d4 in range(4):
                    nc.tensor.matmul(
                        m_ps[d4][:],
                        w_sb[:, c:c + 1],
                        xg[:, d4 * 512:(d4 + 1) * 512],
                        start=(c == 0),
                        stop=(c == CG - 1),
                    )
            m_sb = spool.tile([1, D], f32)
            for d4 in range(4):
                nc.scalar.copy(m_sb[:, d4 * 512:(d4 + 1) * 512], m_ps[d4][:])
            m_bin = dram.tile([1, D], f32)
            m_bout = dram.tile([1, D], f32)
            nc.gpsimd.dma_start(m_bin[:], m_sb[:])
            nc.gpsimd.collective_compute(
                "AllReduce",
                mybir.AluOpType.add,
                replica_groups=[list(range(NC))],
                ins=[m_bin.opt()],
                outs=[m_bout.opt()],
            )
            nc.gpsimd.dma_start(m_d[:], m_bout[:])
    nc.compile()
    return nc


def _program(name, builder):
    if name not in _programs:
        _programs[name] = builder()
    return _programs[name]


def kernel(x, W1, b1, W2, b2):
    x = np.ascontiguousarray(np.asarray(x), dtype=np.float32)
    W1 = np.ascontiguousarray(np.asarray(W1), dtype=np.float32)
    b1 = np.asarray(b1, dtype=np.float32).reshape(H)
    W2 = np.asarray(W2, dtype=np.float32).reshape(1, H)
    b2v = float(np.asarray(b2, dtype=np.float32).reshape(1)[0])

    # ---- L1: scores on device ----
    l1 = _program("l1", _build_l1)
    w1t = np.ascontiguousarray(W1.T)              # [D, H]
    b1c = np.ascontiguousarray(b1.reshape(H, 1))
    w2c = np.ascontiguousarray(W2.T)              # [H, 1]
    xT = x.T                                      # view [D, N]
    in1 = []
    for j in range(NC):
        sh = np.zeros((D, PAD), np.float32)
        sh[:, :SHARD] = xT[:, j * SHARD:(j + 1) * SHARD]
        in1.append({"xt": sh, "w1t": w1t, "b1c": b1c, "w2c": w2c})
    res1 = run_bass_kernel_spmd(l1, in1, list(range(NC)))
    LAST["l1"] = res1
    A = np.concatenate(
        [res1.results[j]["a_out"][0, :SHARD] for j in range(NC)]
    ).astype(np.float64)
    A += b2v

    # ---- host: exact top-k membership (fp64 re-score near the cut), Ak ----
    order = np.argsort(-A, kind="stable")
    tau = A[order[K_TOP - 1]]
    MARGIN = 6e-3   # ~40 sigma of the float32r score error
    cand = np.where(np.abs(A - tau) <= MARGIN)[0]
    if cand.size:
        xa = x[cand].astype(np.float64)
        A[cand] = (
            np.tanh(xa @ W1.T.astype(np.float64) + b1.astype(np.float64))
            @ W2[0].astype(np.float64) + b2v
        )
        order = np.argsort(-A, kind="stable")
    sel = order[:K_TOP]
    e = np.exp(A[sel] - A.max())
    w_norm = e / e.sum()
    Ak = w_norm.astype(np.float32).reshape(K_TOP, 1)

    # ---- L2: gather + weighted sum + AllReduce on device ----
    in2 = []
    overflow = False
    for j in range(NC):
        mask = (sel >= j * SHARD) & (sel < (j + 1) * SHARD)
        li = (sel[mask] - j * SHARD).astype(np.int32)
        lw = w_norm[mask].astype(np.float32)
        if li.size > KG:
            overflow = True
            break
        idx = np.zeros((KG, 1), np.int32)
        wv = np.zeros((KG, 1), np.float32)
        idx[:li.size, 0] = li
        wv[:li.size, 0] = lw
        in2.append({"xs": x[j * SHARD:(j + 1) * SHARD], "gidx": idx, "gw": wv})
    if overflow:
        # pathological selection imbalance (not reachable for iid inputs):
        # keep correctness with a host-side weighted sum
        M = (w_norm[:, None] * x[sel].astype(np.float64)).sum(axis=0).astype(np.float32)
        return (M, Ak)
    l2 = _program("l2", _build_l2)
    res2 = run_bass_kernel_spmd(l2, in2, list(range(NC)))
    LAST["l2"] = res2
    M = np.ascontiguousarray(res2.results[0]["m_out"][0], dtype=np.float32)
    return (M, Ak)
